# revision 13
# baseline (speedup 1.0000x reference)
"""Trainium2 Bass kernel for DeformAxialDW.

Reference computes: out = x + convH(x) + convW(x) where convH/convW are
depthwise 1D convs (7 taps) along H/W with fractional dilation r realized
as bilinear sampling. Expanding the bilinear interpolation over integer
shifts, each conv becomes a per-channel banded (Toeplitz) conv with
2S+1 integer taps, S = floor(3*r)+1.

Per-core plan (1 batch item per NeuronCore, 8 cores):
  - layout: h on SBUF partitions, w in free dim; x split into two aligned
    112-row blocks (rows 0:112 and 112:224), one pair of tiles per channel
  - H-conv: per-channel banded Toeplitz stationary (bf16) x moving (bf16)
    matmuls into fp32 PSUM; cross-block halo handled by "edge" matmuls
    whose Toeplitz is zero except a small corner
  - W-conv: PE-transpose 112x112 blocks of x, then matmul with the
    transposed block as stationary and the per-channel W-Toeplitz as
    moving, accumulated into the SAME PSUM tile as the H-conv
  - identity (+x): fp32 add on VectorE while copying PSUM->SBUF
  - fp32->bf16 casts on GpSimd, PSUM->SBUF transpose copies on ScalarE
"""

import os
import sys

import numpy as np

sys.path.insert(0, "/opt/trn_rl_repo")

import ml_dtypes

BF16 = ml_dtypes.bfloat16

C, H, W = 128, 224, 224
B = 8
HS = 112  # row-block / h_out / w_in block size

_CACHE = {}


def _tap_coeffs(w_taps: np.ndarray, r_val: float, S: int) -> np.ndarray:
    """Expand 7 fractional-dilation taps into 2S+1 integer-shift coeffs."""
    Cn, K = w_taps.shape
    P = K // 2
    alpha = np.zeros((Cn, 2 * S + 1), dtype=np.float64)
    for i in range(K):
        k_pos = i - P
        delta = np.float32(k_pos) * np.float32(r_val)
        d0 = int(np.floor(delta))
        frac = float(np.float32(delta) - np.float32(d0))
        alpha[:, d0 + S] += (1.0 - frac) * w_taps[:, i].astype(np.float64)
        alpha[:, d0 + 1 + S] += frac * w_taps[:, i].astype(np.float64)
    return alpha


def _banded(alpha: np.ndarray, rows: int, cols: int, diag_off: int, S: int):
    """M[i, c, jj] = alpha[c, (i - jj + diag_off)] where |i-jj+diag_off|<=S."""
    Cn = alpha.shape[0]
    out = np.zeros((rows, Cn, cols), dtype=np.float64)
    i = np.arange(rows)[:, None]
    jj = np.arange(cols)[None, :]
    d = i - jj + diag_off
    mask = np.abs(d) <= S
    ii, jjj = np.nonzero(mask)
    out[ii, :, jjj] = alpha[:, d[ii, jjj] + S].T
    return out


def _build_nc(S: int):
    import concourse.mybir as mybir
    from concourse import bacc
    from concourse.tile import TileContext

    f32 = mybir.dt.float32
    bf16 = mybir.dt.bfloat16

    nc = bacc.Bacc("TRN2", target_bir_lowering=False, debug=False)
    x_p = nc.declare_dram_parameter("x", [C, H, W], f32, isOutput=False)
    # gh sections along last dim: [0:HS]=main diag, [HS:2HS]=edge for t=0
    # (h_in block 1 -> h_out block 0), [2HS:3HS]=edge for t=1
    gh_p = nc.declare_dram_parameter("gh", [HS, C, 3 * HS], bf16, isOutput=False)
    gw_p = nc.declare_dram_parameter("gw", [HS, C, HS + 3 * S], bf16, isOutput=False)
    id_p = nc.declare_dram_parameter("ident", [HS, HS], bf16, isOutput=False)
    out_p = nc.declare_dram_parameter("out", [C, H, W], f32, isOutput=True)

    G = 16  # channels per DMA group
    with TileContext(nc) as tc:
        with tc.tile_pool(name="const", bufs=1) as constp, \
             tc.tile_pool(name="xf", bufs=2) as xfp, \
             tc.tile_pool(name="xb", bufs=2) as xbp, \
             tc.tile_pool(name="gt", bufs=2) as gtp, \
             tc.tile_pool(name="xt", bufs=6) as xtp, \
             tc.tile_pool(name="outs", bufs=3) as outp, \
             tc.tile_pool(name="pp", bufs=2, space="PSUM") as ppp, \
             tc.tile_pool(name="po", bufs=6, space="PSUM") as pop:
            ident = constp.tile([HS, HS], bf16)
            nc.sync.dma_start(out=ident[:, :], in_=id_p[:, :])
            for c0 in range(0, C, G):
                ghg = gtp.tile([HS, G, 3 * HS], bf16, tag="gh")
                gwg = gtp.tile([HS, G, HS + 3 * S], bf16, tag="gw")
                nc.sync.dma_start(out=ghg[:, :, :], in_=gh_p[:, c0:c0 + G, :])
                nc.sync.dma_start(out=gwg[:, :, :], in_=gw_p[:, c0:c0 + G, :])
                xf = []
                xb = []
                for t in (0, 1):
                    xf_t = xfp.tile([HS, G, W], f32, tag=f"xf{t}")
                    nc.sync.dma_start(
                        out=xf_t[:, :, :],
                        in_=x_p[c0:c0 + G, t * HS:(t + 1) * HS, :].rearrange(
                            "c h w -> h c w"
                        ),
                    )
                    xb_t = xbp.tile([HS, G, W], bf16, tag=f"xb{t}")
                    nc.gpsimd.tensor_copy(out=xb_t[:, :, :], in_=xf_t[:, :, :])
                    xf.append(xf_t)
                    xb.append(xb_t)
                og0 = outp.tile([HS, G, W], f32, tag="ot0")
                og1 = outp.tile([HS, G, W], f32, tag="ot1")
                og = [og0, og1]
                for cl in range(G):
                    # transpose x blocks: xts[q][:, t, :] = x[tblock_t, wchunk_q].T
                    xts = []
                    for q in (0, 1):
                        xt_t = xtp.tile([HS, 2, HS], bf16, tag=f"xt{q}")
                        pp = ppp.tile([HS, 2, HS], bf16)
                        for t in (0, 1):
                            nc.tensor.matmul(
                                out=pp[:, t, :],
                                lhsT=xb[t][0:HS, cl, q * HS:(q + 1) * HS],
                                rhs=ident[:, :],
                                is_transpose=True,
                                skip_group_check=True,
                            )
                        nc.scalar.copy(out=xt_t[:, :, :], in_=pp[:, :, :])
                        xts.append(xt_t)
                    for t in (0, 1):
                        po = pop.tile([HS, W], f32)
                        # H-conv: main (same-block) + edge (other block)
                        nc.tensor.matmul(
                            out=po[:, :],
                            lhsT=ghg[0:HS, cl, 0:HS],
                            rhs=xb[t][0:HS, cl, :],
                            start=True, stop=False,
                        )
                        nc.tensor.matmul(
                            out=po[:, :],
                            lhsT=ghg[0:HS, cl, (1 + t) * HS:(2 + t) * HS],
                            rhs=xb[1 - t][0:HS, cl, :],
                            start=False, stop=False,
                        )
                        # W-conv: two w_in chunks
                        nc.tensor.matmul(
                            out=po[0:HS, 0:HS + S],
                            lhsT=xts[0][0:HS, t, :],
                            rhs=gwg[0:HS, cl, 2 * S:3 * S + HS],
                            start=False, stop=False,
                        )
                        nc.tensor.matmul(
                            out=po[0:HS, HS - S:W],
                            lhsT=xts[1][0:HS, t, :],
                            rhs=gwg[0:HS, cl, S:2 * S + HS],
                            start=False, stop=True,
                        )
                        nc.vector.tensor_add(
                            out=og[t][:, cl, :], in0=xf[t][0:HS, cl, :], in1=po[:, :]
                        )
                for t in (0, 1):
                    nc.sync.dma_start(
                        out=out_p[c0:c0 + G, t * HS:(t + 1) * HS, :].rearrange(
                            "c h w -> h c w"
                        ),
                        in_=og[t][:, :, :],
                    )
    nc.compile()
    return nc


def _prepare_consts(weight_h, weight_w, r):
    r_val = float(max(np.float32(r), np.float32(1.0)))
    S = int(np.floor(3.0 * r_val)) + 1
    assert S <= 16, f"dilation r={r_val} too large for this kernel (S={S})"
    wh = np.asarray(weight_h)[:, 0, :, 0].astype(np.float64)
    ww = np.asarray(weight_w)[:, 0, 0, :].astype(np.float64)
    ah = _tap_coeffs(wh, r_val, S)
    aw = _tap_coeffs(ww, r_val, S)
    gh = np.concatenate(
        [
            _banded(ah, HS, HS, 0, S),       # main diagonal block
            _banded(ah, HS, HS, HS, S),      # edge: h_in block1 -> h_out block0
            _banded(ah, HS, HS, -HS, S),     # edge: h_in block0 -> h_out block1
        ],
        axis=2,
    ).astype(BF16)
    gw = _banded(aw, HS, HS + 3 * S, 2 * S, S).astype(BF16)
    ident = np.eye(HS, dtype=BF16)
    return S, gh, gw, ident


def kernel(x, weight_h, weight_w, r):
    from concourse.bass_utils import run_bass_kernel_spmd

    x = np.asarray(x, dtype=np.float32)
    assert x.shape == (B, C, H, W), x.shape
    S, gh, gw, ident = _prepare_consts(weight_h, weight_w, r)

    if S not in _CACHE:
        _CACHE[S] = _build_nc(S)
    nc = _CACHE[S]

    in_maps = [
        {"x": x[b], "gh": gh, "gw": gw, "ident": ident} for b in range(B)
    ]
    res = run_bass_kernel_spmd(nc, in_maps, core_ids=list(range(B)))
    out = np.stack([res.results[b]["out"] for b in range(B)], axis=0)
    return out
